# revision 1
# baseline (speedup 1.0000x reference)
"""GraphConv + BatchNorm + LeakyReLU fused layer on 8 Trainium2 NeuronCores.

Strategy (per the node/edge-partition sharding):
  - Edges are bucketed on the host by destination node; destination nodes are
    sharded across the 8 cores (6250 nodes each, padded to 49 blocks of 128).
  - Each core gathers the bf16 source features for its edges straight from HBM
    with dma_gather (int16 indices => x is split in two 25024-row halves), 128
    edges per gather slot.
  - The per-destination-block segment sum is a PE matmul: aggT = G.T @ S where
    G is a [128 edges x 128 feat] gathered tile and S[e, d] = (dst[e] == d) is
    built on DVE with a broadcast iota compare. Accumulated over the block's
    slots in PSUM, this yields agg^T = [feat x dst] directly.
  - x1^T = W_rel^T.T @ aggT + W_root^T.T @ x_own^T accumulates in PSUM;
    leaky_relu is algebraically folded into the next matmul:
        W_lin @ leaky(v) = (0.01 W_lin) @ v + (0.99 W_lin) @ relu(v)
    with v = x1 + b_rel produced by the scalar engine (Identity/Relu, bias as a
    per-partition AP).
  - Batch-norm statistics (sum, sum of squares over nodes) reduce along the
    free axis per gather chunk, are all-reduced across the 8 cores via a DRAM
    bounce buffer; the final per-feature affine + leaky_relu runs batched in
    place on the [feat x node] activations, and each block is transposed back
    to [node x feat] on the PE before the store.
  - Gathers alternate between two SWDGE queues so descriptor generation for
    the next chunk overlaps the previous chunk's SDMA transfer; part of each
    block's S build runs on the scalar engine (relu(1-(iota-dval)^2)) to
    balance DVE vs ACT occupancy.

kernel(**inputs) takes the full-size numpy inputs and returns the full
[50000, 128] float32 output; everything device-side runs SPMD on cores 0-7.
"""
import sys

if "/opt/trn_rl_repo" not in sys.path:
    sys.path.insert(0, "/opt/trn_rl_repo")

import numpy as np
import ml_dtypes

import concourse.bass as bass
import concourse.mybir as mybir
import concourse.tile as tile
from concourse import bacc
from concourse import bass_utils
from concourse.masks import make_identity

F32 = mybir.dt.float32
BF16 = mybir.dt.bfloat16
I16 = mybir.dt.int16

N_NODES = 50000
N_CORES = 8
NPC = N_NODES // N_CORES          # 6250 nodes per core
NBLK = (NPC + 127) // 128         # 49 dst blocks of 128 per core
NPC_PAD = NBLK * 128              # 6272
ROWS_PAD = ((N_NODES + 127) // 128 + 1) * 128  # unused slack is fine
HALF = 25024                      # split point (multiple of 128, < 2**15)
CHUNK = 3                         # dst blocks gathered per dma_gather pair
BN_EPS = 1e-5
NEG = 0.01


def _wrap_idx(idx):
    """int16 gather index layout: i -> [i % 16, i // 16], replicated on all
    8 sixteen-partition groups."""
    n = len(idx)
    assert n % 16 == 0
    w = idx.reshape(n // 16, 16).T
    return np.tile(w, (8, 1))


def _chunks(nblk, chunk):
    out = []
    b = 0
    while b < nblk:
        out.append((b, min(chunk, nblk - b)))
        b += chunk
    return out


def build_program(cfg):
    """Build the SPMD Bass program. cfg keys: n_cores, nblk, l_max, h_max,
    chunk, rows_pad, half, idx_lo_cols, idx_hi_cols."""
    ncores = cfg["n_cores"]
    nblk = cfg["nblk"]
    L = cfg["l_max"]
    Hh = cfg["h_max"]
    chunk = cfg["chunk"]
    rows_pad = cfg["rows_pad"]
    half = cfg["half"]
    npc_pad = nblk * 128
    nslot_blk = L + Hh
    chunks = _chunks(nblk, chunk)
    nchunks = len(chunks)

    nc = bacc.Bacc("TRN2", target_bir_lowering=False, debug=False,
                   num_devices=ncores, num_swdge_queues=2)

    xb_d = nc.dram_tensor("xb", [rows_pad, 128], BF16, kind="ExternalInput")
    xot_d = nc.dram_tensor("x_ownT", [128, npc_pad], BF16, kind="ExternalInput")
    il_d = nc.dram_tensor("idx_lo", [128, cfg["idx_lo_cols"]], I16,
                          kind="ExternalInput")
    ih_d = nc.dram_tensor("idx_hi", [128, cfg["idx_hi_cols"]], I16,
                          kind="ExternalInput")
    dv_d = nc.dram_tensor("dvals", [128, nblk * nslot_blk], BF16,
                          kind="ExternalInput")
    io_d = nc.dram_tensor("iota", [128, 128], BF16, kind="ExternalInput")
    wr_d = nc.dram_tensor("WrT", [128, 128], BF16, kind="ExternalInput")
    wo_d = nc.dram_tensor("WoT", [128, 128], BF16, kind="ExternalInput")
    wa_d = nc.dram_tensor("WlTa", [128, 128], BF16, kind="ExternalInput")
    wb_d = nc.dram_tensor("WlTb", [128, 128], BF16, kind="ExternalInput")
    br_d = nc.dram_tensor("brel", [128, 1], F32, kind="ExternalInput")
    ga_d = nc.dram_tensor("gamma", [128, 1], F32, kind="ExternalInput")
    be_d = nc.dram_tensor("beta", [128, 1], F32, kind="ExternalInput")
    out_d = nc.dram_tensor("out", [npc_pad, 128], F32, kind="ExternalOutput")

    inv_n = 1.0 / float(cfg["n_total"])

    with tile.TileContext(nc) as tc:
        with (
            tc.tile_pool(name="consts", bufs=1) as consts,
            tc.tile_pool(name="gp", bufs=3) as gp,
            tc.tile_pool(name="idxp", bufs=2) as idxp,
            tc.tile_pool(name="sp", bufs=3) as sp,
            tc.tile_pool(name="ps", bufs=4, space="PSUM") as ps,
            tc.tile_pool(name="tp", bufs=2, space="PSUM") as tp,
            tc.tile_pool(name="misc", bufs=3) as misc,
            tc.tile_pool(name="big", bufs=1) as big,
            tc.tile_pool(name="dram", bufs=1, space="DRAM") as dram,
        ):
            # ---- constants / persistent tiles ----
            dv_s = consts.tile([128, nblk * nslot_blk], BF16)
            io_s = consts.tile([128, 128], BF16)
            wr_s = consts.tile([128, 128], BF16)
            wo_s = consts.tile([128, 128], BF16)
            wa_s = consts.tile([128, 128], BF16)
            wb_s = consts.tile([128, 128], BF16)
            br_s = consts.tile([128, 1], F32)
            ga_s = consts.tile([128, 1], F32)
            be_s = consts.tile([128, 1], F32)
            ident = consts.tile([128, 128], F32)
            xot_s = big.tile([128, npc_pad], BF16)
            x3_s = big.tile([128, npc_pad], F32)
            sums = big.tile([128, nchunks], F32)
            sqs = big.tile([128, nchunks], F32)

            nc.sync.dma_start(dv_s[:], dv_d[:])
            nc.scalar.dma_start(io_s[:], io_d[:])
            nc.scalar.dma_start(wr_s[:], wr_d[:])
            nc.scalar.dma_start(wo_s[:], wo_d[:])
            nc.scalar.dma_start(wa_s[:], wa_d[:])
            nc.scalar.dma_start(wb_s[:], wb_d[:])
            nc.scalar.dma_start(br_s[:], br_d[:])
            nc.scalar.dma_start(ga_s[:], ga_d[:])
            nc.scalar.dma_start(be_s[:], be_d[:])
            nc.scalar.dma_start(xot_s[:], xot_d[:])
            make_identity(nc, ident[:])
            ndv = big.tile([128, nblk * nslot_blk], BF16)
            nc.vector.tensor_scalar_mul(ndv[:], dv_s[:], -1.0)
            ones1 = consts.tile([128, 1], F32)
            nc.vector.memset(ones1[:], 1.0)

            io_ap = io_s[:]

            ilo_col = 0
            ihi_col = 0
            for ci, (b0, nb) in enumerate(chunks):
                n_lo = nb * L * 128
                n_hi = nb * Hh * 128
                G_lo = gp.tile([128, chunk * L, 128], BF16, tag="Glo")
                G_hi = gp.tile([128, chunk * Hh, 128], BF16, tag="Ghi")
                ilo_t = idxp.tile([128, chunk * L * 8], I16, tag="ilo")
                ihi_t = idxp.tile([128, chunk * Hh * 8], I16, tag="ihi")
                nc.sync.dma_start(ilo_t[:, 0:n_lo // 16],
                                  il_d[:, ilo_col:ilo_col + n_lo // 16])
                nc.sync.dma_start(ihi_t[:, 0:n_hi // 16],
                                  ih_d[:, ihi_col:ihi_col + n_hi // 16])
                ilo_col += n_lo // 16
                ihi_col += n_hi // 16
                nc.gpsimd.dma_gather(
                    out_ap=G_lo[:, 0:nb * L, :],
                    in_ap=xb_d[0:half, :],
                    idxs_ap=ilo_t[:, 0:n_lo // 16],
                    num_idxs=n_lo,
                    num_idxs_reg=n_lo,
                    elem_size=128,
                    single_packet=False,
                    queue_num=0,
                )
                nc.gpsimd.dma_gather(
                    out_ap=G_hi[:, 0:nb * Hh, :],
                    in_ap=xb_d[half:rows_pad, :],
                    idxs_ap=ihi_t[:, 0:n_hi // 16],
                    num_idxs=n_hi,
                    num_idxs_reg=n_hi,
                    elem_size=128,
                    single_packet=False,
                    queue_num=1,
                )

                slot0 = b0 * nslot_blk  # dvals column base for this chunk
                for b in range(nb):
                    blk = b0 + b
                    # ---- S tiles (is_equal against broadcast iota) ----
                    S_lo = sp.tile([128, L, 128], BF16, tag="slo")
                    S_hi = sp.tile([128, Hh, 128], BF16, tag="shi")
                    dl = slot0 + b * L
                    dh = slot0 + nb * L + b * Hh
                    iota_lo = bass.AP(tensor=io_ap.tensor, offset=io_ap.offset,
                                      ap=[io_ap.ap[0], [0, L], io_ap.ap[1]])
                    iota_hi = bass.AP(tensor=io_ap.tensor, offset=io_ap.offset,
                                      ap=[io_ap.ap[0], [0, Hh], io_ap.ap[1]])
                    dvl = dv_s[:, dl:dl + L]
                    dvh = dv_s[:, dh:dh + Hh]
                    dvl_bc = bass.AP(tensor=dvl.tensor, offset=dvl.offset,
                                     ap=[dvl.ap[0], dvl.ap[1], [0, 128]])
                    dvh_bc = bass.AP(tensor=dvh.tensor, offset=dvh.offset,
                                     ap=[dvh.ap[0], dvh.ap[1], [0, 128]])
                    nc.vector.tensor_tensor(out=S_lo[:], in0=iota_lo,
                                            in1=dvl_bc,
                                            op=mybir.AluOpType.is_equal)
                    act_hi = min(6, Hh - 1)
                    dve_hi = Hh - act_hi
                    iota_hi2 = bass.AP(tensor=io_ap.tensor,
                                       offset=io_ap.offset,
                                       ap=[io_ap.ap[0], [0, dve_hi],
                                           io_ap.ap[1]])
                    dvh2 = dv_s[:, dh:dh + dve_hi]
                    dvh2_bc = bass.AP(tensor=dvh2.tensor, offset=dvh2.offset,
                                      ap=[dvh2.ap[0], dvh2.ap[1], [0, 128]])
                    nc.vector.tensor_tensor(out=S_hi[:, 0:dve_hi, :],
                                            in0=iota_hi2, in1=dvh2_bc,
                                            op=mybir.AluOpType.is_equal)
                    for t in range(dve_hi, Hh):
                        # S = relu(1 - (iota - dval)^2), exact for integers
                        z_t = misc.tile([128, 128], BF16, tag="z")
                        nc.scalar.activation(
                            z_t[:], io_s[:],
                            mybir.ActivationFunctionType.Square,
                            bias=ndv[:, dh + t:dh + t + 1], scale=1.0)
                        nc.scalar.activation(
                            S_hi[:, t, :], z_t[:],
                            mybir.ActivationFunctionType.Relu,
                            bias=ones1[:], scale=-1.0)

                    # ---- segment-sum matmuls: aggT[c, d] in PSUM ----
                    agg_ps = ps.tile([128, 128], F32, tag="ps")
                    for t in range(L):
                        nc.tensor.matmul(agg_ps[:], lhsT=G_lo[:, b * L + t, :],
                                         rhs=S_lo[:, t, :],
                                         start=(t == 0), stop=False)
                    for t in range(Hh):
                        nc.tensor.matmul(agg_ps[:],
                                         lhsT=G_hi[:, b * Hh + t, :],
                                         rhs=S_hi[:, t, :],
                                         start=False, stop=(t == Hh - 1))
                    aggT = misc.tile([128, 128], BF16, tag="aggT")
                    nc.scalar.copy(aggT[:], agg_ps[:])

                    # ---- x1^T = W_rel^T.T @ aggT + W_root^T.T @ x_own^T ----
                    x1_ps = ps.tile([128, 128], F32, tag="ps")
                    nc.tensor.matmul(x1_ps[:], lhsT=wr_s[:], rhs=aggT[:],
                                     start=True, stop=False)
                    nc.tensor.matmul(x1_ps[:], lhsT=wo_s[:],
                                     rhs=xot_s[:, blk * 128:(blk + 1) * 128],
                                     start=False, stop=True)

                    # v = x1 + b_rel ; r = relu(v) (both bf16, scalar engine)
                    v_t = misc.tile([128, 128], BF16, tag="v")
                    r_t = misc.tile([128, 128], BF16, tag="r")
                    nc.scalar.activation(v_t[:], x1_ps[:],
                                         mybir.ActivationFunctionType.Identity,
                                         bias=br_s[:], scale=1.0)
                    nc.scalar.activation(r_t[:], x1_ps[:],
                                         mybir.ActivationFunctionType.Relu,
                                         bias=br_s[:], scale=1.0)

                    # x3^T = (0.01 W_lin)^T.T @ v + (0.99 W_lin)^T.T @ r
                    x3_ps = ps.tile([128, 128], F32, tag="ps")
                    nc.tensor.matmul(x3_ps[:], lhsT=wa_s[:], rhs=v_t[:],
                                     start=True, stop=False)
                    nc.tensor.matmul(x3_ps[:], lhsT=wb_s[:], rhs=r_t[:],
                                     start=False, stop=True)
                    nc.scalar.copy(x3_s[:, blk * 128:(blk + 1) * 128],
                                   x3_ps[:])

                # ---- per-chunk statistics over this chunk's x3 region ----
                if b0 + nb == nblk and cfg["n_own"] < npc_pad:
                    # zero padded node columns before they enter statistics
                    nc.vector.memset(x3_s[:, cfg["n_own"]:npc_pad], 0.0)
                xch = x3_s[:, b0 * 128:(b0 + nb) * 128]
                nc.vector.tensor_reduce(sums[:, ci:ci + 1], xch,
                                        axis=mybir.AxisListType.X,
                                        op=mybir.AluOpType.add)
                junk = misc.tile([128, chunk * 128], F32, tag="sqscr")
                nc.scalar.activation(junk[:, 0:nb * 128], xch,
                                     mybir.ActivationFunctionType.Square,
                                     accum_out=sqs[:, ci:ci + 1])

            # ---- global BN statistics via AllReduce ----
            stat2 = consts.tile([128, 2], F32)
            nc.vector.tensor_reduce(stat2[:, 0:1], sums[:],
                                    axis=mybir.AxisListType.X,
                                    op=mybir.AluOpType.add)
            nc.vector.tensor_reduce(stat2[:, 1:2], sqs[:],
                                    axis=mybir.AxisListType.X,
                                    op=mybir.AluOpType.add)
            cc_in = dram.tile([128, 2], F32)
            cc_out = dram.tile([128, 2], F32)
            nc.gpsimd.dma_start(cc_in[:], stat2[:])
            if ncores > 1 and not cfg.get("no_cc"):
                nc.gpsimd.collective_compute(
                    "AllReduce",
                    mybir.AluOpType.add,
                    replica_groups=[list(range(ncores))],
                    ins=[cc_in[:].opt()],
                    outs=[cc_out[:].opt()],
                )
                red = cc_out
            else:
                red = cc_in
            stat_r = consts.tile([128, 2], F32)
            nc.sync.dma_start(stat_r[:], red[:])

            mean = consts.tile([128, 1], F32)
            ex2 = consts.tile([128, 1], F32)
            var = consts.tile([128, 1], F32)
            rstd = consts.tile([128, 1], F32)
            scl = consts.tile([128, 1], F32)
            bia = consts.tile([128, 1], F32)
            tmp1 = consts.tile([128, 1], F32)
            nc.vector.tensor_scalar_mul(mean[:], stat_r[:, 0:1], inv_n)
            nc.vector.tensor_scalar_mul(ex2[:], stat_r[:, 1:2], inv_n)
            nc.vector.tensor_tensor(out=tmp1[:], in0=mean[:], in1=mean[:],
                                    op=mybir.AluOpType.mult)
            nc.vector.tensor_sub(var[:], ex2[:], tmp1[:])
            # rstd = 1/sqrt(var + eps)
            epsv = consts.tile([128, 1], F32)
            nc.vector.memset(epsv[:], BN_EPS)
            nc.scalar.activation(rstd[:], var[:],
                                 mybir.ActivationFunctionType.Sqrt,
                                 bias=epsv[:], scale=1.0)
            nc.vector.reciprocal(rstd[:], rstd[:])
            nc.vector.tensor_tensor(out=scl[:], in0=ga_s[:], in1=rstd[:],
                                    op=mybir.AluOpType.mult)
            nc.vector.tensor_tensor(out=tmp1[:], in0=mean[:], in1=scl[:],
                                    op=mybir.AluOpType.mult)
            nc.vector.tensor_sub(bia[:], be_s[:], tmp1[:])

            # ---- normalize + leaky (batched, in place), transpose, store
            nc.scalar.activation(x3_s[:], x3_s[:],
                                 mybir.ActivationFunctionType.Identity,
                                 bias=bia[:], scale=scl[:])
            nc.vector.scalar_tensor_tensor(
                out=x3_s[:], in0=x3_s[:], scalar=NEG, in1=x3_s[:],
                op0=mybir.AluOpType.mult, op1=mybir.AluOpType.max)
            for blk in range(nblk):
                x3_blk = x3_s[:, blk * 128:(blk + 1) * 128]
                tr_ps = ps.tile([128, 128], F32, tag="ps")
                nc.tensor.transpose(tr_ps[:], x3_blk, ident[:])
                o_sb = misc.tile([128, 128], F32, tag="osb")
                nc.vector.tensor_copy(o_sb[:], tr_ps[:])
                nc.sync.dma_start(out_d[blk * 128:(blk + 1) * 128, :],
                                  o_sb[:])

    nc.compile()
    return nc


def preprocess(x, edge_index, cfg):
    """Host-side sharding: returns per-core input dicts (without weights)."""
    ncores = cfg["n_cores"]
    nblk = cfg["nblk"]
    chunk = cfg["chunk"]
    half = cfg["half"]
    rows_pad = cfg["rows_pad"]
    npc = cfg["npc"]
    npc_pad = nblk * 128
    n = x.shape[0]

    src = np.asarray(edge_index[0], dtype=np.int64)
    dst = np.asarray(edge_index[1], dtype=np.int64)

    core = dst // npc
    loc = dst - core * npc
    blk = loc // 128
    dloc = loc % 128
    hi = (src >= half).astype(np.int64)

    # group edges by (core, blk, half); stable order within groups
    key = (core * nblk + blk) * 2 + hi
    order = np.argsort(key, kind="stable")
    key_s = key[order]
    src_s = src[order]
    dloc_s = dloc[order]
    ngroups = ncores * nblk * 2
    counts = np.bincount(key_s, minlength=ngroups)
    starts = np.zeros(ngroups + 1, dtype=np.int64)
    np.cumsum(counts, out=starts[1:])

    cl = counts.reshape(ncores, nblk, 2)
    l_max = int(np.ceil(cl[:, :, 0].max() / 128)) if cl[:, :, 0].max() else 1
    h_max = int(np.ceil(cl[:, :, 1].max() / 128)) if cl[:, :, 1].max() else 1
    cfg["l_max"] = max(l_max, 1)
    cfg["h_max"] = max(h_max, 1)
    L, Hh = cfg["l_max"], cfg["h_max"]

    # padded per-(core, blk, half) edge arrays
    idx_pad = np.zeros((ncores, nblk, 2, max(L, Hh) * 128), dtype=np.int64)
    dv_pad = np.full((ncores, nblk, 2, max(L, Hh) * 128), 255, dtype=np.int64)
    pos = np.arange(len(src_s)) - starts[key_s]
    c_e = key_s // (nblk * 2)
    b_e = (key_s // 2) % nblk
    h_e = key_s % 2
    idx_pad[c_e, b_e, h_e, pos] = src_s - h_e * half
    dv_pad[c_e, b_e, h_e, pos] = dloc_s

    xb = np.zeros((rows_pad, 128), dtype=ml_dtypes.bfloat16)
    xb[:n] = x.astype(ml_dtypes.bfloat16)

    chunks = _chunks(nblk, chunk)
    per_core = []
    for c in range(ncores):
        il_parts, ih_parts = [], []
        dv = np.full((128, nblk * (L + Hh)), 255, dtype=np.int64)
        for (b0, nb) in chunks:
            lo_cat = idx_pad[c, b0:b0 + nb, 0, :L * 128].reshape(-1)
            hi_cat = idx_pad[c, b0:b0 + nb, 1, :Hh * 128].reshape(-1)
            il_parts.append(_wrap_idx(lo_cat))
            ih_parts.append(_wrap_idx(hi_cat))
            s0 = b0 * (L + Hh)
            # dvals slot s = chunk-local: lo slots then hi slots, block-major
            dvlo = dv_pad[c, b0:b0 + nb, 0, :L * 128].reshape(nb * L, 128).T
            dvhi = dv_pad[c, b0:b0 + nb, 1, :Hh * 128].reshape(nb * Hh, 128).T
            dv[:, s0:s0 + nb * L] = dvlo
            dv[:, s0 + nb * L:s0 + nb * (L + Hh)] = dvhi
        row0 = c * npc
        xoT = np.zeros((128, npc_pad), dtype=ml_dtypes.bfloat16)
        hi_row = min(row0 + npc_pad, n)
        xoT[:, :hi_row - row0] = xb[row0:hi_row].T
        per_core.append({
            "xb": xb,
            "x_ownT": xoT,
            "idx_lo": np.concatenate(il_parts, axis=1).astype(np.int16),
            "idx_hi": np.concatenate(ih_parts, axis=1).astype(np.int16),
            "dvals": dv.astype(ml_dtypes.bfloat16),
        })
    cfg["idx_lo_cols"] = per_core[0]["idx_lo"].shape[1]
    cfg["idx_hi_cols"] = per_core[0]["idx_hi"].shape[1]
    return per_core


_PROGRAM_CACHE = {}


def run(x, edge_index, W_rel, b_rel, W_root, W_lin, b_lin, gamma, beta, cfg):
    per_core = preprocess(x, edge_index, cfg)

    iota = np.tile(np.arange(128, dtype=np.float32), (128, 1))
    shared = {
        "iota": iota.astype(ml_dtypes.bfloat16),
        "WrT": np.ascontiguousarray(W_rel.T).astype(ml_dtypes.bfloat16),
        "WoT": np.ascontiguousarray(W_root.T).astype(ml_dtypes.bfloat16),
        "WlTa": np.ascontiguousarray((NEG * W_lin).T).astype(ml_dtypes.bfloat16),
        "WlTb": np.ascontiguousarray(((1.0 - NEG) * W_lin).T).astype(
            ml_dtypes.bfloat16),
        "brel": b_rel.reshape(128, 1).astype(np.float32),
        "gamma": gamma.reshape(128, 1).astype(np.float32),
        "beta": beta.reshape(128, 1).astype(np.float32),
    }
    in_maps = [dict(m, **shared) for m in per_core]

    key = (cfg["n_cores"], cfg["nblk"], cfg["l_max"], cfg["h_max"],
           cfg["chunk"], cfg["idx_lo_cols"], cfg["idx_hi_cols"])
    if key not in _PROGRAM_CACHE:
        _PROGRAM_CACHE[key] = build_program(cfg)
    nc = _PROGRAM_CACHE[key]

    res = bass_utils.run_bass_kernel_spmd(
        nc, in_maps, core_ids=list(range(cfg["n_cores"])))
    n = x.shape[0]
    npc = cfg["npc"]
    out = np.empty((n, 128), dtype=np.float32)
    for c in range(cfg["n_cores"]):
        out[c * npc:(c + 1) * npc] = res.results[c]["out"][:npc]
    return out


def kernel(x, edge_index, batch, W_rel, b_rel, W_root, W_lin, b_lin, gamma,
           beta):
    x = np.asarray(x, dtype=np.float32)
    cfg = {
        "n_cores": N_CORES,
        "npc": NPC,
        "nblk": NBLK,
        "chunk": CHUNK,
        "rows_pad": ROWS_PAD,
        "half": HALF,
        "n_total": N_NODES,
        "n_own": NPC,
    }
    return run(x, np.asarray(edge_index), np.asarray(W_rel, dtype=np.float32),
               np.asarray(b_rel, dtype=np.float32),
               np.asarray(W_root, dtype=np.float32),
               np.asarray(W_lin, dtype=np.float32),
               np.asarray(b_lin, dtype=np.float32),
               np.asarray(gamma, dtype=np.float32),
               np.asarray(beta, dtype=np.float32), cfg)



# revision 5
# speedup vs baseline: 1.0516x; 1.0516x over previous
"""GraphConv + BatchNorm + LeakyReLU fused layer on 8 Trainium2 NeuronCores.

Strategy (node/edge-partition sharding, v2):
  - Destination nodes are sharded across the 8 cores (6250 each). Within a
    core, dst nodes are assigned to 49 blocks of <=128 by balanced (LPT-style)
    packing on in-degree so every block has nearly the same edge count; a
    uniform slot count L = ceil(max_block_edges/128) is baked into the
    program.
  - Each core fetches the bf16 source row for each of its edges with a single
    indirect DMA gather stream (int32 indices, no halving), one 256B
    descriptor per edge, into G tiles of [128 edge-lanes, slots, 128 feat].
  - The per-block segment sum is a PE matmul: aggT += G_t^T @ S_t where
    S_t[lane, d] = (dst_lane == d) is built batched per block on DVE with a
    broadcast iota compare.
  - x1^T = W_rel^T.T @ aggT + W_root^T.T @ x_own^T accumulates in PSUM;
    x2 = leaky_relu(x1) is one DVE scalar_tensor_tensor (max(0.01*v, v))
    straight out of PSUM; x3^T = W_lin^T.T @ x2 is a single matmul.
  - The x3 block copy (ACT) accumulates per-feature sums; a Square activation
    accumulates sums of squares. Global BN stats go through a DRAM bounce +
    AllReduce; the final affine+leaky and a PE transpose per block produce
    bf16 outputs stored per block (upcast to f32 on the host).

kernel(**inputs) takes full-size numpy inputs, returns [50000, 128] float32.
"""
import sys

if "/opt/trn_rl_repo" not in sys.path:
    sys.path.insert(0, "/opt/trn_rl_repo")

import numpy as np
import ml_dtypes

import concourse.bass as bass
import concourse.mybir as mybir
import concourse.tile as tile
from concourse import bacc
from concourse import bass_utils
from concourse.masks import make_identity

F32 = mybir.dt.float32
BF16 = mybir.dt.bfloat16
I32 = mybir.dt.int32

N_NODES = 50000
N_CORES = 8
NPC = N_NODES // N_CORES          # 6250 nodes per core
NBLK = 49                         # dst blocks per core
NPC_PAD = NBLK * 128              # 6272
LAST_BLK = NPC - 128 * (NBLK - 1)  # 106 real nodes in the last block
ROWS_PAD = ((N_NODES + 127) // 128) * 128  # 50048
CHUNK = 4                         # dst blocks per gather
BN_EPS = 1e-5
NEG = 0.01


def _chunks(nblk, chunk):
    out = []
    b = 0
    while b < nblk:
        out.append((b, min(chunk, nblk - b)))
        b += chunk
    return out


def build_program(cfg):
    """Build the SPMD Bass program. cfg keys: n_cores, nblk, l_max, chunk,
    n_total, has_brel, has_blin."""
    ncores = cfg["n_cores"]
    nblk = cfg["nblk"]
    L = cfg["l_max"]
    chunk = cfg["chunk"]
    npc_pad = nblk * 128
    chunks = _chunks(nblk, chunk)
    nslots = nblk * L

    nc = bacc.Bacc("TRN2", target_bir_lowering=False, debug=False,
                   num_devices=ncores)

    xb_d = nc.dram_tensor("xb", [ROWS_PAD, 128], BF16, kind="ExternalInput")
    xot_d = nc.dram_tensor("x_ownT", [128, npc_pad], BF16,
                           kind="ExternalInput")
    ix_d = nc.dram_tensor("idx", [128, nslots], I32, kind="ExternalInput")
    dv_d = nc.dram_tensor("dvals", [128, nslots], BF16, kind="ExternalInput")
    io_d = nc.dram_tensor("iota", [128, 128], BF16, kind="ExternalInput")
    wr_d = nc.dram_tensor("WrT", [128, 128], BF16, kind="ExternalInput")
    wo_d = nc.dram_tensor("WoT", [128, 128], BF16, kind="ExternalInput")
    wl_d = nc.dram_tensor("WlT", [128, 128], BF16, kind="ExternalInput")
    br_d = nc.dram_tensor("brel", [128, 1], F32, kind="ExternalInput")
    bl_d = nc.dram_tensor("blin", [128, 1], F32, kind="ExternalInput")
    ga_d = nc.dram_tensor("gamma", [128, 1], F32, kind="ExternalInput")
    be_d = nc.dram_tensor("beta", [128, 1], F32, kind="ExternalInput")
    out_d = nc.dram_tensor("out", [npc_pad, 128], BF16, kind="ExternalOutput")

    inv_n = 1.0 / float(cfg["n_total"])

    with tile.TileContext(nc) as tc:
        with (
            tc.tile_pool(name="consts", bufs=1) as consts,
            tc.tile_pool(name="gp", bufs=2) as gp,
            tc.tile_pool(name="idxp", bufs=2) as idxp,
            tc.tile_pool(name="sp", bufs=3) as sp,
            tc.tile_pool(name="ps", bufs=2, space="PSUM") as ps,
            tc.tile_pool(name="tp", bufs=2, space="PSUM") as tp,
            tc.tile_pool(name="misc", bufs=3) as misc,
            tc.tile_pool(name="big", bufs=1) as big,
            tc.tile_pool(name="dram", bufs=1, space="DRAM") as dram,
        ):
            # ---- constants / persistent tiles ----
            io_s = consts.tile([128, 128], BF16)
            wr_s = consts.tile([128, 128], BF16)
            wo_s = consts.tile([128, 128], BF16)
            wl_s = consts.tile([128, 128], BF16)
            br_s = consts.tile([128, 1], F32)
            bl_s = consts.tile([128, 1], F32)
            ga_s = consts.tile([128, 1], F32)
            be_s = consts.tile([128, 1], F32)
            ident = consts.tile([128, 128], F32)
            dv_s = consts.tile([128, nslots], BF16)
            xot_s = big.tile([128, npc_pad], BF16)
            x3_s = big.tile([128, npc_pad], F32)
            sums = big.tile([128, nblk + 1], F32)
            sqs = big.tile([128, nblk + 1], F32)

            nc.scalar.dma_start(io_s[:], io_d[:])
            nc.scalar.dma_start(wr_s[:], wr_d[:])
            nc.scalar.dma_start(wo_s[:], wo_d[:])
            nc.scalar.dma_start(wl_s[:], wl_d[:])
            nc.scalar.dma_start(br_s[:], br_d[:])
            nc.scalar.dma_start(bl_s[:], bl_d[:])
            nc.scalar.dma_start(ga_s[:], ga_d[:])
            nc.scalar.dma_start(be_s[:], be_d[:])
            nc.sync.dma_start(dv_s[:], dv_d[:])
            nc.scalar.dma_start(xot_s[:], xot_d[:])
            make_identity(nc, ident[:])

            io_ap = io_s[:]

            for (b0, nb) in chunks:
                ncols = nb * L
                s0 = b0 * L
                ix_t = idxp.tile([128, chunk * L], I32, tag="ix")
                nc.sync.dma_start(ix_t[:, 0:ncols], ix_d[:, s0:s0 + ncols])
                G = gp.tile([128, chunk * L, 128], BF16, tag="G")
                nc.gpsimd.indirect_dma_start(
                    out=G[:, 0:ncols, :],
                    out_offset=None,
                    in_=xb_d[:],
                    in_offset=bass.IndirectOffsetOnAxis(
                        ap=ix_t[:, 0:ncols], axis=0),
                )

                for b in range(nb):
                    blk = b0 + b
                    # ---- S tile (is_equal against broadcast iota) ----
                    S = sp.tile([128, L, 128], BF16, tag="S")
                    dvb = dv_s[:, blk * L:(blk + 1) * L]
                    iota_bc = bass.AP(tensor=io_ap.tensor, offset=io_ap.offset,
                                      ap=[io_ap.ap[0], [0, L], io_ap.ap[1]])
                    dv_bc = bass.AP(tensor=dvb.tensor, offset=dvb.offset,
                                    ap=[dvb.ap[0], dvb.ap[1], [0, 128]])
                    nc.vector.tensor_tensor(out=S[:], in0=iota_bc, in1=dv_bc,
                                            op=mybir.AluOpType.is_equal)

                    # ---- segment-sum matmuls: aggT[c, d] in PSUM ----
                    agg_ps = ps.tile([128, 128], F32, tag="agg")
                    for t in range(L):
                        nc.tensor.matmul(agg_ps[:], lhsT=G[:, b * L + t, :],
                                         rhs=S[:, t, :],
                                         start=(t == 0), stop=(t == L - 1))
                    aggT = misc.tile([128, 128], BF16, tag="aggT")
                    nc.scalar.copy(aggT[:], agg_ps[:])

                    # ---- x1^T = W_rel^T.T @ aggT + W_root^T.T @ x_own^T ----
                    x1_ps = ps.tile([128, 128], F32, tag="x1")
                    nc.tensor.matmul(x1_ps[:], lhsT=wr_s[:], rhs=aggT[:],
                                     start=True, stop=False)
                    nc.tensor.matmul(x1_ps[:], lhsT=wo_s[:],
                                     rhs=xot_s[:, blk * 128:(blk + 1) * 128],
                                     start=False, stop=True)

                    # x2 = leaky(x1 + b_rel): PSUM -> SBUF copy (adds b_rel),
                    # then a one-op leaky on the SBUF copy
                    x2_sb = misc.tile([128, 128], BF16, tag="x2")
                    v_sb = misc.tile([128, 128], BF16, tag="v")
                    brel = br_s[:] if cfg["has_brel"] else 0.0
                    nc.scalar.activation(
                        v_sb[:], x1_ps[:],
                        mybir.ActivationFunctionType.Identity,
                        bias=brel, scale=1.0)
                    nc.vector.scalar_tensor_tensor(
                        out=x2_sb[:], in0=v_sb[:], scalar=NEG, in1=v_sb[:],
                        op0=mybir.AluOpType.mult, op1=mybir.AluOpType.max)

                    # x3^T = W_lin^T.T @ x2
                    x3_ps = ps.tile([128, 128], F32, tag="x3")
                    nc.tensor.matmul(x3_ps[:], lhsT=wl_s[:], rhs=x2_sb[:],
                                     start=True, stop=True)

                    # copy to x3_s (+ b_lin) accumulating per-feature sums
                    w = LAST_BLK if blk == nblk - 1 else 128
                    blin = bl_s[:] if cfg["has_blin"] else 0.0
                    nc.scalar.activation(
                        x3_s[:, blk * 128:blk * 128 + w], x3_ps[:, 0:w],
                        mybir.ActivationFunctionType.Identity,
                        bias=blin, scale=1.0,
                        accum_out=sums[:, blk:blk + 1])
                    junk = misc.tile([128, 128], BF16, tag="junk")
                    nc.scalar.activation(
                        junk[:, 0:w], x3_s[:, blk * 128:blk * 128 + w],
                        mybir.ActivationFunctionType.Square,
                        accum_out=sqs[:, blk:blk + 1])

            if NPC < npc_pad:
                nc.vector.memset(x3_s[:, NPC:npc_pad], 0.0)

            # ---- global BN statistics via AllReduce ----
            stat2 = consts.tile([128, 2], F32)
            nc.vector.tensor_reduce(stat2[:, 0:1], sums[:, 0:nblk],
                                    axis=mybir.AxisListType.X,
                                    op=mybir.AluOpType.add)
            nc.vector.tensor_reduce(stat2[:, 1:2], sqs[:, 0:nblk],
                                    axis=mybir.AxisListType.X,
                                    op=mybir.AluOpType.add)
            cc_in = dram.tile([128, 2], F32)
            cc_out = dram.tile([128, 2], F32)
            nc.gpsimd.dma_start(cc_in[:], stat2[:])
            if ncores > 1 and not cfg.get("no_cc"):
                nc.gpsimd.collective_compute(
                    "AllReduce",
                    mybir.AluOpType.add,
                    replica_groups=[list(range(ncores))],
                    ins=[cc_in[:].opt()],
                    outs=[cc_out[:].opt()],
                )
                red = cc_out
            else:
                red = cc_in
            stat_r = consts.tile([128, 2], F32)
            nc.sync.dma_start(stat_r[:], red[:])

            mean = consts.tile([128, 1], F32)
            ex2 = consts.tile([128, 1], F32)
            var = consts.tile([128, 1], F32)
            rstd = consts.tile([128, 1], F32)
            scl = consts.tile([128, 1], F32)
            bia = consts.tile([128, 1], F32)
            tmp1 = consts.tile([128, 1], F32)
            nc.vector.tensor_scalar_mul(mean[:], stat_r[:, 0:1], inv_n)
            nc.vector.tensor_scalar_mul(ex2[:], stat_r[:, 1:2], inv_n)
            nc.vector.tensor_tensor(out=tmp1[:], in0=mean[:], in1=mean[:],
                                    op=mybir.AluOpType.mult)
            nc.vector.tensor_sub(var[:], ex2[:], tmp1[:])
            epsv = consts.tile([128, 1], F32)
            nc.vector.memset(epsv[:], BN_EPS)
            nc.scalar.activation(rstd[:], var[:],
                                 mybir.ActivationFunctionType.Sqrt,
                                 bias=epsv[:], scale=1.0)
            nc.vector.reciprocal(rstd[:], rstd[:])
            nc.vector.tensor_tensor(out=scl[:], in0=ga_s[:], in1=rstd[:],
                                    op=mybir.AluOpType.mult)
            nc.vector.tensor_tensor(out=tmp1[:], in0=mean[:], in1=scl[:],
                                    op=mybir.AluOpType.mult)
            nc.vector.tensor_sub(bia[:], be_s[:], tmp1[:])

            # ---- normalize + leaky (batched), transpose, store bf16 ----
            QN = 4
            qblks = (nblk + QN - 1) // QN
            done = 0
            for q in range(QN):
                bq = min(qblks, nblk - done)
                if bq <= 0:
                    break
                lo = done * 128
                hi = (done + bq) * 128
                nc.scalar.activation(x3_s[:, lo:hi], x3_s[:, lo:hi],
                                     mybir.ActivationFunctionType.Identity,
                                     bias=bia[:], scale=scl[:])
                nc.vector.scalar_tensor_tensor(
                    out=x3_s[:, lo:hi], in0=x3_s[:, lo:hi], scalar=NEG,
                    in1=x3_s[:, lo:hi],
                    op0=mybir.AluOpType.mult, op1=mybir.AluOpType.max)
                for blk in range(done, done + bq):
                    x3_blk = x3_s[:, blk * 128:(blk + 1) * 128]
                    tr_ps = tp.tile([128, 128], F32, tag="tr")
                    nc.tensor.transpose(tr_ps[:], x3_blk, ident[:])
                    o_sb = misc.tile([128, 128], BF16, tag="osb")
                    nc.vector.tensor_copy(o_sb[:], tr_ps[:])
                    nc.sync.dma_start(out_d[blk * 128:(blk + 1) * 128, :],
                                      o_sb[:])
                done += bq

    nc.compile()
    return nc


def preprocess(x, edge_index, cfg):
    """Host-side sharding: balanced dst blocks + per-core edge/index arrays.

    Returns (per_core_inputs, perm) where perm[c] maps padded slot positions
    to global node ids (for unpermuting the output on the host).
    """
    ncores = cfg["n_cores"]
    nblk = cfg["nblk"]
    n = x.shape[0]
    npc = cfg["npc"]
    npc_pad = nblk * 128

    src = np.asarray(edge_index[0], dtype=np.int64)
    dst = np.asarray(edge_index[1], dtype=np.int64)
    core = dst // npc
    loc = dst - core * npc

    xb = np.zeros((ROWS_PAD, 128), dtype=ml_dtypes.bfloat16)
    xb[:n] = x.astype(ml_dtypes.bfloat16)

    # per-(core, loc) degree
    deg = np.zeros((ncores, npc), dtype=np.int64)
    np.add.at(deg, (core, loc), 1)

    blk_of = np.empty((ncores, npc), dtype=np.int64)
    pos_of = np.empty((ncores, npc), dtype=np.int64)
    caps = np.full(nblk, 128, dtype=np.int64)
    caps[nblk - 1] = LAST_BLK
    l_need = 0
    for c in range(ncores):
        order = np.argsort(-deg[c], kind="stable")
        sums_b = np.zeros(nblk, dtype=np.int64)
        fill = np.zeros(nblk, dtype=np.int64)
        # serpentine round-robin over capacity-open blocks (near-LPT for
        # smooth degree distributions, vectorized per round)
        ptr = 0
        rnd = 0
        while ptr < npc:
            open_b = np.where(fill < caps)[0]
            k = min(len(open_b), npc - ptr)
            open_b = open_b[:k]
            # alternate direction by round, weakest-sum-first placement
            ob = open_b[np.argsort(sums_b[open_b], kind="stable")]
            nodes = order[ptr:ptr + k]
            blk_of[c, nodes] = ob
            pos_of[c, nodes] = fill[ob]
            sums_b[ob] += deg[c, nodes]
            fill[ob] += 1
            ptr += k
            rnd += 1
        l_need = max(l_need, int(np.ceil(sums_b.max() / 128)))
    cfg["l_max"] = max(l_need, 1)
    L = cfg["l_max"]
    nslots = nblk * L

    # per-edge placement
    e_blk = blk_of[core, loc]
    e_dloc = pos_of[core, loc]
    key = core * nblk + e_blk
    order_e = np.argsort(key, kind="stable")
    key_s = key[order_e]
    src_s = src[order_e]
    dloc_s = e_dloc[order_e]
    ngroups = ncores * nblk
    counts = np.bincount(key_s, minlength=ngroups)
    starts = np.zeros(ngroups + 1, dtype=np.int64)
    np.cumsum(counts, out=starts[1:])
    j = np.arange(len(src_s)) - starts[key_s]   # rank within (core, blk)
    c_e = key_s // nblk
    b_e = key_s % nblk

    idx_all = np.zeros((ncores, nblk, L * 128), dtype=np.int32)
    dv_all = np.full((ncores, nblk, L * 128), 255, dtype=np.int64)
    idx_all[c_e, b_e, j] = src_s
    dv_all[c_e, b_e, j] = dloc_s

    per_core = []
    perm = np.full((ncores, npc_pad), -1, dtype=np.int64)
    for c in range(ncores):
        # idx layout: [128 lanes, nblk*L slots]; edge j in (c, b) sits at
        # slot b*L + j//128, lane j%128
        iw = idx_all[c].reshape(nblk, L, 128)          # [b, t, lane]
        dw = dv_all[c].reshape(nblk, L, 128)
        idx = np.ascontiguousarray(
            iw.transpose(2, 0, 1).reshape(128, nslots))
        dv = np.ascontiguousarray(
            dw.transpose(2, 0, 1).reshape(128, nslots))
        # own nodes, transposed, in permuted order
        nodes = np.arange(npc, dtype=np.int64)
        slot = blk_of[c] * 128 + pos_of[c]
        perm[c, slot] = nodes + c * npc
        xoT = np.zeros((128, npc_pad), dtype=ml_dtypes.bfloat16)
        xoT[:, slot] = xb[nodes + c * npc].T
        per_core.append({
            "xb": xb,
            "x_ownT": xoT,
            "idx": idx,
            "dvals": dv.astype(ml_dtypes.bfloat16),
        })
    return per_core, perm


_PROGRAM_CACHE = {}


def run(x, edge_index, W_rel, b_rel, W_root, W_lin, b_lin, gamma, beta, cfg):
    per_core, perm = preprocess(x, edge_index, cfg)
    cfg["has_brel"] = bool(np.any(b_rel != 0))
    cfg["has_blin"] = bool(np.any(b_lin != 0))

    iota = np.tile(np.arange(128, dtype=np.float32), (128, 1))
    shared = {
        "iota": iota.astype(ml_dtypes.bfloat16),
        "WrT": np.ascontiguousarray(W_rel.T).astype(ml_dtypes.bfloat16),
        "WoT": np.ascontiguousarray(W_root.T).astype(ml_dtypes.bfloat16),
        "WlT": np.ascontiguousarray(W_lin.T).astype(ml_dtypes.bfloat16),
        "brel": b_rel.reshape(128, 1).astype(np.float32),
        "blin": b_lin.reshape(128, 1).astype(np.float32),
        "gamma": gamma.reshape(128, 1).astype(np.float32),
        "beta": beta.reshape(128, 1).astype(np.float32),
    }
    in_maps = [dict(m, **shared) for m in per_core]

    key = (cfg["n_cores"], cfg["nblk"], cfg["l_max"], cfg["chunk"],
           cfg["has_brel"], cfg["has_blin"])
    if key not in _PROGRAM_CACHE:
        _PROGRAM_CACHE[key] = build_program(cfg)
    nc = _PROGRAM_CACHE[key]

    res = bass_utils.run_bass_kernel_spmd(
        nc, in_maps, core_ids=list(range(cfg["n_cores"])))
    n = x.shape[0]
    out = np.empty((n, 128), dtype=np.float32)
    for c in range(cfg["n_cores"]):
        o = np.asarray(res.results[c]["out"]).astype(np.float32)
        m = perm[c] >= 0
        out[perm[c][m]] = o[m]
    return out


def kernel(x, edge_index, batch, W_rel, b_rel, W_root, W_lin, b_lin, gamma,
           beta):
    x = np.asarray(x, dtype=np.float32)
    cfg = {
        "n_cores": N_CORES,
        "npc": NPC,
        "nblk": NBLK,
        "chunk": CHUNK,
        "n_total": N_NODES,
    }
    return run(x, np.asarray(edge_index), np.asarray(W_rel, dtype=np.float32),
               np.asarray(b_rel, dtype=np.float32),
               np.asarray(W_root, dtype=np.float32),
               np.asarray(W_lin, dtype=np.float32),
               np.asarray(b_lin, dtype=np.float32),
               np.asarray(gamma, dtype=np.float32),
               np.asarray(beta, dtype=np.float32), cfg)


# revision 8
# speedup vs baseline: 1.0766x; 1.0238x over previous
"""GraphConv + BatchNorm + LeakyReLU fused layer on 8 Trainium2 NeuronCores.

Strategy (node/edge-partition sharding, v2):
  - Destination nodes are sharded across the 8 cores (6250 each). Within a
    core, dst nodes are assigned to 98 blocks of width 64 by balanced
    (LPT-style) packing on in-degree so every block has nearly the same edge
    count; a uniform slot count L = ceil(max_block_edges/128) is baked into
    the program.
  - Each core fetches the bf16 source row for each of its edges with a single
    indirect DMA gather stream (int32 indices, no halving), one 256B
    descriptor per edge, into G tiles of [128 edge-lanes, slots, 128 feat].
  - The per-block segment sum is a PE matmul: aggT += G_t^T @ S_t where
    S_t[lane, d] = (dst_lane == d) is a [128, 64] one-hot built batched per
    block on DVE with a broadcast iota compare (64-wide S halves DVE work
    vs 128-wide blocks).
  - x1^T = W_rel^T.T @ aggT + W_root^T.T @ x_own^T accumulates in PSUM;
    x2 = leaky_relu(x1 + b_rel) is an ACT copy (bias) + one DVE
    scalar_tensor_tensor (max(0.01*v, v)); x3^T = W_lin^T.T @ x2.
  - The x3 block copy (ACT) accumulates per-feature sums; a Square activation
    accumulates sums of squares. Global BN stats go through a DRAM bounce +
    AllReduce; the final affine+leaky and a PE transpose per block-pair
    produce bf16 outputs stored per 128 rows (upcast to f32 on the host).

kernel(**inputs) takes full-size numpy inputs, returns [50000, 128] float32.
"""
import sys

if "/opt/trn_rl_repo" not in sys.path:
    sys.path.insert(0, "/opt/trn_rl_repo")

import numpy as np
import ml_dtypes

import concourse.bass as bass
import concourse.mybir as mybir
import concourse.tile as tile
from concourse import bacc
from concourse import bass_utils
from concourse.masks import make_identity

F32 = mybir.dt.float32
BF16 = mybir.dt.bfloat16
I32 = mybir.dt.int32

N_NODES = 50000
N_CORES = 8
NPC = N_NODES // N_CORES          # 6250 nodes per core
BW = 64                           # dst-block width (S matrix width)
NBLK = (NPC + BW - 1) // BW       # 98 dst blocks per core
NPC_PAD = NBLK * BW               # 6272
LAST_BLK = NPC - BW * (NBLK - 1)  # 42 real nodes in the last block
ROWS_PAD = ((N_NODES + 127) // 128) * 128  # 50048
CHUNK = 8                         # dst blocks per gather
BN_EPS = 1e-5
NEG = 0.01


def _chunks(nblk, chunk):
    out = []
    b = 0
    while b < nblk:
        out.append((b, min(chunk, nblk - b)))
        b += chunk
    return out


def build_program(cfg):
    """Build the SPMD Bass program. cfg keys: n_cores, nblk, l_max, chunk,
    n_total, has_brel, has_blin."""
    ncores = cfg["n_cores"]
    nblk = cfg["nblk"]
    L = cfg["l_max"]
    chunk = cfg["chunk"]
    npc_pad = nblk * BW
    chunks = _chunks(nblk, chunk)
    nslots = nblk * L

    nc = bacc.Bacc("TRN2", target_bir_lowering=False, debug=False,
                   num_devices=ncores)

    xb_d = nc.dram_tensor("xb", [ROWS_PAD, 128], BF16, kind="ExternalInput")
    xot_d = nc.dram_tensor("x_ownT", [128, npc_pad], BF16,
                           kind="ExternalInput")
    ix_d = nc.dram_tensor("idx", [128, nslots], I32, kind="ExternalInput")
    dv_d = nc.dram_tensor("dvals", [128, nslots], BF16, kind="ExternalInput")
    io_d = nc.dram_tensor("iota", [128, 128], BF16, kind="ExternalInput")
    wr_d = nc.dram_tensor("WrT", [128, 128], BF16, kind="ExternalInput")
    wo_d = nc.dram_tensor("WoT", [128, 128], BF16, kind="ExternalInput")
    wl_d = nc.dram_tensor("WlT", [128, 128], BF16, kind="ExternalInput")
    br_d = nc.dram_tensor("brel", [128, 1], F32, kind="ExternalInput")
    bl_d = nc.dram_tensor("blin", [128, 1], F32, kind="ExternalInput")
    ga_d = nc.dram_tensor("gamma", [128, 1], F32, kind="ExternalInput")
    be_d = nc.dram_tensor("beta", [128, 1], F32, kind="ExternalInput")
    out_d = nc.dram_tensor("out", [npc_pad, 128], BF16, kind="ExternalOutput")

    inv_n = 1.0 / float(cfg["n_total"])

    with tile.TileContext(nc) as tc:
        with (
            tc.tile_pool(name="consts", bufs=1) as consts,
            tc.tile_pool(name="gp", bufs=2) as gp,
            tc.tile_pool(name="idxp", bufs=2) as idxp,
            tc.tile_pool(name="sp", bufs=4) as sp,
            tc.tile_pool(name="ps", bufs=2, space="PSUM") as ps,
            tc.tile_pool(name="tp", bufs=2, space="PSUM") as tp,
            tc.tile_pool(name="misc", bufs=4) as misc,
            tc.tile_pool(name="big", bufs=1) as big,
            tc.tile_pool(name="dram", bufs=1, space="DRAM") as dram,
        ):
            # ---- constants / persistent tiles ----
            io_s = consts.tile([128, 128], BF16)
            wr_s = consts.tile([128, 128], BF16)
            wo_s = consts.tile([128, 128], BF16)
            wl_s = consts.tile([128, 128], BF16)
            br_s = consts.tile([128, 1], F32)
            bl_s = consts.tile([128, 1], F32)
            ga_s = consts.tile([128, 1], F32)
            be_s = consts.tile([128, 1], F32)
            ident = consts.tile([128, 128], F32)
            dv_s = consts.tile([128, nslots], BF16)
            xot_s = big.tile([128, npc_pad], BF16)
            x3_s = big.tile([128, npc_pad], F32)
            sums = big.tile([128, nblk + 2], F32)
            sqs = big.tile([128, nblk + 2], F32)

            nc.scalar.dma_start(io_s[:], io_d[:])
            nc.scalar.dma_start(wr_s[:], wr_d[:])
            nc.scalar.dma_start(wo_s[:], wo_d[:])
            nc.scalar.dma_start(wl_s[:], wl_d[:])
            nc.scalar.dma_start(br_s[:], br_d[:])
            nc.scalar.dma_start(bl_s[:], bl_d[:])
            nc.scalar.dma_start(ga_s[:], ga_d[:])
            nc.scalar.dma_start(be_s[:], be_d[:])
            nc.sync.dma_start(dv_s[:], dv_d[:])
            nc.scalar.dma_start(xot_s[:], xot_d[:])
            make_identity(nc, ident[:])

            io_ap = io_s[:, 0:BW]

            for (b0, nb) in chunks:
                ncols = nb * L
                s0 = b0 * L
                ix_t = idxp.tile([128, chunk * L], I32, tag="ix")
                nc.sync.dma_start(ix_t[:, 0:ncols], ix_d[:, s0:s0 + ncols])
                G = gp.tile([128, chunk * L, 128], BF16, tag="G")
                nc.gpsimd.indirect_dma_start(
                    out=G[:, 0:ncols, :],
                    out_offset=None,
                    in_=xb_d[:],
                    in_offset=bass.IndirectOffsetOnAxis(
                        ap=ix_t[:, 0:ncols], axis=0),
                )

                for b in range(nb):
                    blk = b0 + b
                    # ---- S tile (is_equal against broadcast iota) ----
                    S = sp.tile([128, L, BW], BF16, tag="S")
                    dvb = dv_s[:, blk * L:(blk + 1) * L]
                    iota_bc = bass.AP(tensor=io_ap.tensor, offset=io_ap.offset,
                                      ap=[io_ap.ap[0], [0, L], io_ap.ap[1]])
                    dv_bc = bass.AP(tensor=dvb.tensor, offset=dvb.offset,
                                    ap=[dvb.ap[0], dvb.ap[1], [0, BW]])
                    nc.vector.tensor_tensor(out=S[:], in0=iota_bc, in1=dv_bc,
                                            op=mybir.AluOpType.is_equal)

                    # ---- segment-sum matmuls: aggT[c, d] in PSUM ----
                    agg_ps = ps.tile([128, BW], F32, tag="agg")
                    for t in range(L):
                        nc.tensor.matmul(agg_ps[:], lhsT=G[:, b * L + t, :],
                                         rhs=S[:, t, :],
                                         start=(t == 0), stop=(t == L - 1))
                    aggT = misc.tile([128, BW], BF16, tag="aggT")
                    nc.scalar.copy(aggT[:], agg_ps[:])

                    # ---- x1^T = W_rel^T.T @ aggT + W_root^T.T @ x_own^T ----
                    x1_ps = ps.tile([128, BW], F32, tag="x1")
                    nc.tensor.matmul(x1_ps[:], lhsT=wr_s[:], rhs=aggT[:],
                                     start=True, stop=False)
                    nc.tensor.matmul(x1_ps[:], lhsT=wo_s[:],
                                     rhs=xot_s[:, blk * BW:(blk + 1) * BW],
                                     start=False, stop=True)

                    # x2 = leaky(x1 + b_rel): PSUM -> SBUF copy (adds b_rel),
                    # then a one-op leaky on the SBUF copy
                    x2_sb = misc.tile([128, BW], BF16, tag="x2")
                    v_sb = misc.tile([128, BW], BF16, tag="v")
                    brel = br_s[:] if cfg["has_brel"] else 0.0
                    nc.scalar.activation(
                        v_sb[:], x1_ps[:],
                        mybir.ActivationFunctionType.Identity,
                        bias=brel, scale=1.0)
                    nc.vector.scalar_tensor_tensor(
                        out=x2_sb[:], in0=v_sb[:], scalar=NEG, in1=v_sb[:],
                        op0=mybir.AluOpType.mult, op1=mybir.AluOpType.max)

                    # x3^T = W_lin^T.T @ x2
                    x3_ps = ps.tile([128, BW], F32, tag="x3")
                    nc.tensor.matmul(x3_ps[:], lhsT=wl_s[:], rhs=x2_sb[:],
                                     start=True, stop=True)

                    # copy to x3_s (+ b_lin) accumulating per-feature sums
                    w = LAST_BLK if blk == nblk - 1 else BW
                    blin = bl_s[:] if cfg["has_blin"] else 0.0
                    nc.scalar.activation(
                        x3_s[:, blk * BW:blk * BW + w], x3_ps[:, 0:w],
                        mybir.ActivationFunctionType.Identity,
                        bias=blin, scale=1.0,
                        accum_out=sums[:, blk:blk + 1])
                    junk = misc.tile([128, BW], BF16, tag="junk")
                    nc.scalar.activation(
                        junk[:, 0:w], x3_s[:, blk * BW:blk * BW + w],
                        mybir.ActivationFunctionType.Square,
                        accum_out=sqs[:, blk:blk + 1])

            if NPC < npc_pad:
                nc.vector.memset(x3_s[:, NPC:npc_pad], 0.0)

            # ---- global BN statistics via AllReduce ----
            stat2 = consts.tile([128, 2], F32)
            nc.vector.tensor_reduce(stat2[:, 0:1], sums[:, 0:nblk],
                                    axis=mybir.AxisListType.X,
                                    op=mybir.AluOpType.add)
            nc.vector.tensor_reduce(stat2[:, 1:2], sqs[:, 0:nblk],
                                    axis=mybir.AxisListType.X,
                                    op=mybir.AluOpType.add)
            cc_in = dram.tile([128, 2], F32)
            cc_out = dram.tile([128, 2], F32)
            nc.gpsimd.dma_start(cc_in[:], stat2[:])
            if ncores > 1 and not cfg.get("no_cc"):
                nc.gpsimd.collective_compute(
                    "AllReduce",
                    mybir.AluOpType.add,
                    replica_groups=[list(range(ncores))],
                    ins=[cc_in[:].opt()],
                    outs=[cc_out[:].opt()],
                )
                red = cc_out
            else:
                red = cc_in
            stat_r = consts.tile([128, 2], F32)
            nc.sync.dma_start(stat_r[:], red[:])

            mean = consts.tile([128, 1], F32)
            ex2 = consts.tile([128, 1], F32)
            var = consts.tile([128, 1], F32)
            rstd = consts.tile([128, 1], F32)
            scl = consts.tile([128, 1], F32)
            bia = consts.tile([128, 1], F32)
            tmp1 = consts.tile([128, 1], F32)
            nc.vector.tensor_scalar_mul(mean[:], stat_r[:, 0:1], inv_n)
            nc.vector.tensor_scalar_mul(ex2[:], stat_r[:, 1:2], inv_n)
            nc.vector.tensor_tensor(out=tmp1[:], in0=mean[:], in1=mean[:],
                                    op=mybir.AluOpType.mult)
            nc.vector.tensor_sub(var[:], ex2[:], tmp1[:])
            epsv = consts.tile([128, 1], F32)
            nc.vector.memset(epsv[:], BN_EPS)
            nc.scalar.activation(rstd[:], var[:],
                                 mybir.ActivationFunctionType.Sqrt,
                                 bias=epsv[:], scale=1.0)
            nc.vector.reciprocal(rstd[:], rstd[:])
            nc.vector.tensor_tensor(out=scl[:], in0=ga_s[:], in1=rstd[:],
                                    op=mybir.AluOpType.mult)
            nc.vector.tensor_tensor(out=tmp1[:], in0=mean[:], in1=scl[:],
                                    op=mybir.AluOpType.mult)
            nc.vector.tensor_sub(bia[:], be_s[:], tmp1[:])

            # ---- normalize + leaky (batched), transpose, store bf16 ----
            npairs = npc_pad // 128
            QN = 4
            qpairs = (npairs + QN - 1) // QN
            done = 0
            for q in range(QN):
                pq = min(qpairs, npairs - done)
                if pq <= 0:
                    break
                lo = done * 128
                hi = (done + pq) * 128
                nc.scalar.activation(x3_s[:, lo:hi], x3_s[:, lo:hi],
                                     mybir.ActivationFunctionType.Identity,
                                     bias=bia[:], scale=scl[:])
                nc.vector.scalar_tensor_tensor(
                    out=x3_s[:, lo:hi], in0=x3_s[:, lo:hi], scalar=NEG,
                    in1=x3_s[:, lo:hi],
                    op0=mybir.AluOpType.mult, op1=mybir.AluOpType.max)
                for p in range(done, done + pq):
                    x3_blk = x3_s[:, p * 128:(p + 1) * 128]
                    tr_ps = tp.tile([128, 128], F32, tag="tr")
                    nc.tensor.transpose(tr_ps[:], x3_blk, ident[:])
                    o_sb = misc.tile([128, 128], BF16, tag="osb")
                    nc.vector.tensor_copy(o_sb[:], tr_ps[:])
                    nc.sync.dma_start(out_d[p * 128:(p + 1) * 128, :],
                                      o_sb[:])
                done += pq

    nc.compile()
    return nc


def preprocess(x, edge_index, cfg):
    """Host-side sharding: balanced dst blocks + per-core edge/index arrays.

    Returns (per_core_inputs, perm) where perm[c] maps padded slot positions
    to global node ids (for unpermuting the output on the host).
    """
    ncores = cfg["n_cores"]
    nblk = cfg["nblk"]
    n = x.shape[0]
    npc = cfg["npc"]
    npc_pad = nblk * BW

    src = np.asarray(edge_index[0], dtype=np.int64)
    dst = np.asarray(edge_index[1], dtype=np.int64)
    core = dst // npc
    loc = dst - core * npc

    xb = np.zeros((ROWS_PAD, 128), dtype=ml_dtypes.bfloat16)
    xb[:n] = x.astype(ml_dtypes.bfloat16)

    # per-(core, loc) degree
    deg = np.zeros((ncores, npc), dtype=np.int64)
    np.add.at(deg, (core, loc), 1)

    blk_of = np.empty((ncores, npc), dtype=np.int64)
    pos_of = np.empty((ncores, npc), dtype=np.int64)
    caps = np.full(nblk, BW, dtype=np.int64)
    caps[nblk - 1] = LAST_BLK
    l_need = 0
    for c in range(ncores):
        order = np.argsort(-deg[c], kind="stable")
        sums_b = np.zeros(nblk, dtype=np.int64)
        fill = np.zeros(nblk, dtype=np.int64)
        # greedy rounds: place the next batch of highest-degree nodes onto
        # the currently-lightest capacity-open blocks (near-LPT)
        ptr = 0
        while ptr < npc:
            open_b = np.where(fill < caps)[0]
            k = min(len(open_b), npc - ptr)
            ob = open_b[np.argsort(sums_b[open_b], kind="stable")[:k]]
            nodes = order[ptr:ptr + k]
            blk_of[c, nodes] = ob
            pos_of[c, nodes] = fill[ob]
            sums_b[ob] += deg[c, nodes]
            fill[ob] += 1
            ptr += k
        l_need = max(l_need, int(np.ceil(sums_b.max() / 128)))
    cfg["l_max"] = max(l_need, 1)
    L = cfg["l_max"]
    nslots = nblk * L

    # per-edge placement
    e_blk = blk_of[core, loc]
    e_dloc = pos_of[core, loc]
    key = core * nblk + e_blk
    order_e = np.argsort(key, kind="stable")
    key_s = key[order_e]
    src_s = src[order_e]
    dloc_s = e_dloc[order_e]
    ngroups = ncores * nblk
    counts = np.bincount(key_s, minlength=ngroups)
    starts = np.zeros(ngroups + 1, dtype=np.int64)
    np.cumsum(counts, out=starts[1:])
    j = np.arange(len(src_s)) - starts[key_s]   # rank within (core, blk)
    c_e = key_s // nblk
    b_e = key_s % nblk

    idx_all = np.zeros((ncores, nblk, L * 128), dtype=np.int32)
    dv_all = np.full((ncores, nblk, L * 128), 255, dtype=np.int64)
    idx_all[c_e, b_e, j] = src_s
    dv_all[c_e, b_e, j] = dloc_s

    per_core = []
    perm = np.full((ncores, npc_pad), -1, dtype=np.int64)
    for c in range(ncores):
        # idx layout: [128 lanes, nblk*L slots]; edge j in (c, b) sits at
        # slot b*L + j//128, lane j%128
        iw = idx_all[c].reshape(nblk, L, 128)          # [b, t, lane]
        dw = dv_all[c].reshape(nblk, L, 128)
        idx = np.ascontiguousarray(
            iw.transpose(2, 0, 1).reshape(128, nslots))
        dv = np.ascontiguousarray(
            dw.transpose(2, 0, 1).reshape(128, nslots))
        # own nodes, transposed, in permuted order
        nodes = np.arange(npc, dtype=np.int64)
        slot = blk_of[c] * BW + pos_of[c]
        perm[c, slot] = nodes + c * npc
        xoT = np.zeros((128, npc_pad), dtype=ml_dtypes.bfloat16)
        xoT[:, slot] = xb[nodes + c * npc].T
        per_core.append({
            "xb": xb,
            "x_ownT": xoT,
            "idx": idx,
            "dvals": dv.astype(ml_dtypes.bfloat16),
        })
    return per_core, perm


_PROGRAM_CACHE = {}


def run(x, edge_index, W_rel, b_rel, W_root, W_lin, b_lin, gamma, beta, cfg):
    per_core, perm = preprocess(x, edge_index, cfg)
    cfg["has_brel"] = bool(np.any(b_rel != 0))
    cfg["has_blin"] = bool(np.any(b_lin != 0))

    iota = np.tile(np.arange(128, dtype=np.float32), (128, 1))
    shared = {
        "iota": iota.astype(ml_dtypes.bfloat16),
        "WrT": np.ascontiguousarray(W_rel.T).astype(ml_dtypes.bfloat16),
        "WoT": np.ascontiguousarray(W_root.T).astype(ml_dtypes.bfloat16),
        "WlT": np.ascontiguousarray(W_lin.T).astype(ml_dtypes.bfloat16),
        "brel": b_rel.reshape(128, 1).astype(np.float32),
        "blin": b_lin.reshape(128, 1).astype(np.float32),
        "gamma": gamma.reshape(128, 1).astype(np.float32),
        "beta": beta.reshape(128, 1).astype(np.float32),
    }
    in_maps = [dict(m, **shared) for m in per_core]

    key = (cfg["n_cores"], cfg["nblk"], cfg["l_max"], cfg["chunk"],
           cfg["has_brel"], cfg["has_blin"])
    if key not in _PROGRAM_CACHE:
        _PROGRAM_CACHE[key] = build_program(cfg)
    nc = _PROGRAM_CACHE[key]

    res = bass_utils.run_bass_kernel_spmd(
        nc, in_maps, core_ids=list(range(cfg["n_cores"])))
    n = x.shape[0]
    out = np.empty((n, 128), dtype=np.float32)
    for c in range(cfg["n_cores"]):
        o = np.asarray(res.results[c]["out"]).astype(np.float32)
        m = perm[c] >= 0
        out[perm[c][m]] = o[m]
    return out


def kernel(x, edge_index, batch, W_rel, b_rel, W_root, W_lin, b_lin, gamma,
           beta):
    x = np.asarray(x, dtype=np.float32)
    cfg = {
        "n_cores": N_CORES,
        "npc": NPC,
        "nblk": NBLK,
        "chunk": CHUNK,
        "n_total": N_NODES,
    }
    return run(x, np.asarray(edge_index), np.asarray(W_rel, dtype=np.float32),
               np.asarray(b_rel, dtype=np.float32),
               np.asarray(W_root, dtype=np.float32),
               np.asarray(W_lin, dtype=np.float32),
               np.asarray(b_lin, dtype=np.float32),
               np.asarray(gamma, dtype=np.float32),
               np.asarray(beta, dtype=np.float32), cfg)
